# revision 94
# baseline (speedup 1.0000x reference)
"""Trainium2 Bass kernel for nn_DeltaNet_19430432047178 — T-sharded.

Strategy (8 cores, SPMD):
  - sequence-parallel: core c owns tokens [c*256, (c+1)*256)
  - conv window + 3-token mini-window via per-core host inputs (no
    collective); gate's G[t-1] column computed from the mini-window
  - qkv/rope/phi local; v projected directly in [token, vdim]
    orientation (no transposes); chunked linear attention (2 chunks of
    128) with cross-core KV prefix via ONE bf16 AllGather of per-core
    state sums + masked (per-core mask8 input) prefix reduction
  - LayerNorm fully local; MoE gate local; gate weights applied at the
    per-expert PSUM consume (with b2 bias folded into the same op)
  - dense MoE: all-expert fp8 weights streamed from HBM in 1MB DMAs
    (w1 on scalar HWDGE, w2 on sync HWDGE); hidden act = scalar
    AF.Silu straight to fp8; expert 0 fully prefetched into SBUF
    under phases 4-5 (wo/wg freed after phase 4 make room)
  - attention matmuls bf16, MoE matmuls fp8 DoubleRow, f32 PSUM
"""
import numpy as np
import ml_dtypes


class _PhaseStop(Exception):
    pass

NC_N = 8
T = 2048
TPC = 256
D = 1024
H = 16
DH = 64
E = 4
HD = 4096
P = 128
CH = 128
KB = 8

BF = ml_dtypes.bfloat16

_PROGRAM = None


def _build_program(limit=99, repeat=1):
    import concourse.mybir as mybir
    import concourse.tile as tile
    from concourse import bacc
    from concourse.masks import make_identity
    import contextlib

    f32 = mybir.dt.float32
    bf16 = mybir.dt.bfloat16
    f8 = mybir.dt.float8e4
    AF = mybir.ActivationFunctionType
    OP = mybir.AluOpType

    nc = bacc.Bacc()

    xw_d = nc.declare_dram_parameter("xw", [P, KB * (TPC + 2)], bf16, isOutput=False)
    hx_d = nc.declare_dram_parameter("hx", [P, KB * 3], bf16, isOutput=False)
    xs32_d = nc.declare_dram_parameter("xs32", [P, KB * TPC], f32, isOutput=False)
    wdw_d = nc.declare_dram_parameter("wdw", [P, KB * 3], f32, isOutput=False)
    wpw_d = nc.declare_dram_parameter("wpw", [P, 64 * P], bf16, isOutput=False)
    wqkv_d = nc.declare_dram_parameter("wqkv", [P, KB * 24 * P], bf16, isOutput=False)
    ctab_d = nc.declare_dram_parameter("ctab", [P, TPC], bf16, isOutput=False)
    stab_d = nc.declare_dram_parameter("stab", [P, TPC], bf16, isOutput=False)
    p64_d = nc.declare_dram_parameter("p64", [P, P], bf16, isOutput=False)
    mask_d = nc.declare_dram_parameter("mask", [P, P], f32, isOutput=False)
    mask8_d = nc.declare_dram_parameter("mask8", [P, 8], f32, isOutput=False)
    sofs_d = nc.declare_dram_parameter("sofs", [P, 1], mybir.dt.int32,
                                       isOutput=False)
    wo_d = nc.declare_dram_parameter("wo", [P, 64 * P], bf16, isOutput=False)
    wg_d = nc.declare_dram_parameter("wg", [P, 64 * P], bf16, isOutput=False)
    bg_d = nc.declare_dram_parameter("bg", [P, 8], f32, isOutput=False)
    lng_d = nc.declare_dram_parameter("lng", [P, 8], f32, isOutput=False)
    lnb_d = nc.declare_dram_parameter("lnb", [P, 8], f32, isOutput=False)
    wmg_d = nc.declare_dram_parameter("wmg", [P, KB * 4], bf16, isOutput=False)
    b1_d = nc.declare_dram_parameter("b1", [P, E * 32], f32, isOutput=False)
    b2x_d = nc.declare_dram_parameter("b2x", [P, E * 8], f32, isOutput=False)
    w1s_d = nc.declare_dram_parameter("w1s", [P, E * 8 * 4096], f8, isOutput=False)
    w2s_d = nc.declare_dram_parameter("w2s", [P, E * 8 * 4096], f8, isOutput=False)
    y_d = nc.declare_dram_parameter("y", [P, KB * TPC], f32, isOutput=True)

    rg = [list(range(NC_N))]
    st_in = nc.dram_tensor("st_in", [P, 8 * 65], bf16)
    st_out = nc.dram_tensor("st_out", [NC_N * P, 8 * 65], bf16, addr_space="Shared")

    _open = []
    with tile.TileContext(nc) as tc:
        stack = contextlib.ExitStack()
        with stack:
            consts = stack.enter_context(tc.tile_pool(name="consts", bufs=1))
            wpw_sb = consts.tile([P, 64 * P], bf16, tag="wpw")
            nc.scalar.dma_start(out=wpw_sb[:], in_=wpw_d[:])
            wqkv_sb = consts.tile([P, KB * 24 * P], bf16, tag="wqkv")
            nc.scalar.dma_start(out=wqkv_sb[:], in_=wqkv_d[:])
            wdw_sb = consts.tile([P, KB * 3], f32, tag="wdw")
            nc.scalar.dma_start(out=wdw_sb[:], in_=wdw_d[:])
            ctab_sb = consts.tile([P, TPC], bf16, tag="ctab")
            nc.scalar.dma_start(out=ctab_sb[:], in_=ctab_d[:])
            stab_sb = consts.tile([P, TPC], bf16, tag="stab")
            nc.scalar.dma_start(out=stab_sb[:], in_=stab_d[:])
            p64_sb = consts.tile([P, P], bf16, tag="p64")
            nc.scalar.dma_start(out=p64_sb[:], in_=p64_d[:])
            mask_sb = consts.tile([P, P], f32, tag="mask")
            nc.scalar.dma_start(out=mask_sb[:], in_=mask_d[:])
            mask8_sb = consts.tile([P, 8], f32, tag="mask8")
            nc.scalar.dma_start(out=mask8_sb[:], in_=mask8_d[:])
            sofs_sb = consts.tile([P, 1], mybir.dt.int32, tag="sofs")
            nc.scalar.dma_start(out=sofs_sb[:], in_=sofs_d[:])
            bg_sb = consts.tile([P, 8], f32, tag="bg")
            nc.scalar.dma_start(out=bg_sb[:], in_=bg_d[:])
            lng_sb = consts.tile([P, 8], f32, tag="lng")
            nc.scalar.dma_start(out=lng_sb[:], in_=lng_d[:])
            lnb_sb = consts.tile([P, 8], f32, tag="lnb")
            nc.scalar.dma_start(out=lnb_sb[:], in_=lnb_d[:])
            wmg_sb = consts.tile([P, KB * 4], bf16, tag="wmg")
            nc.scalar.dma_start(out=wmg_sb[:], in_=wmg_d[:])
            b1_sb = consts.tile([P, E * 32], f32, tag="b1")
            nc.scalar.dma_start(out=b1_sb[:], in_=b1_d[:])
            b2x_sb = consts.tile([P, E * 8], f32, tag="b2x")
            nc.scalar.dma_start(out=b2x_sb[:], in_=b2x_d[:])
            ident_sb = consts.tile([P, P], bf16, tag="ident")
            make_identity(nc, ident_sb[:])
            ones128 = consts.tile([P, 1], bf16, tag="ones128")
            nc.vector.memset(ones128[:], 1.0)
            ones4 = consts.tile([4, 1], bf16, tag="ones4")
            nc.vector.memset(ones4[:], 1.0)
            ones1f = consts.tile([1, P], f32, tag="ones1f")
            nc.vector.memset(ones1f[:], 1.0)
            eps1 = consts.tile([1, 1], f32, tag="eps1")
            nc.vector.memset(eps1[:], 1e-5)
            stream = stack.enter_context(tc.tile_pool(name="stream", bufs=2))

            def rep_body():
                # persistent-through-MoE activations
                stB = contextlib.ExitStack()
                _open.append(stB)
                actB = stB.enter_context(tc.tile_pool(name="actB", bufs=1))
                x2s32 = actB.tile([P, KB * TPC], f32, tag="x2s32")
                hbf = actB.tile([P, KB * TPC], bf16, tag="hbf")
                hf8 = actB.tile([P, KB * TPC], f8, tag="hf8")
                gwb = [actB.tile([P, TPC], f32, tag=f"gwb{e}", name=f"gwb{e}")
                       for e in range(E)]
                # wo/wg weights, dead after phase 4 (freed for MoE prefetch)
                stW = contextlib.ExitStack()
                _open.append(stW)
                wowg = stW.enter_context(tc.tile_pool(name="wowg", bufs=1))
                wo_sb = wowg.tile([P, 64 * P], bf16, tag="wo")
                nc.sync.dma_start(out=wo_sb[:], in_=wo_d[:])
                wg_sb = wowg.tile([P, 64 * P], bf16, tag="wg")
                nc.sync.dma_start(out=wg_sb[:], in_=wg_d[:])
                # activations dead after wo/gate
                stA = contextlib.ExitStack()
                _open.append(stA)
                actA1 = stA.enter_context(tc.tile_pool(name="actA1", bufs=1))
                x1s32 = actA1.tile([P, KB * TPC], f32, tag="x1s32")
                x1bf = actA1.tile([P, KB * (TPC + 1)], bf16, tag="x1bf")

                def open_actA2():
                    return stA.enter_context(tc.tile_pool(name="actA2", bufs=1))

                # ========== Phase 1: conv mixer ==========
                with tc.tile_pool(name="pconv", bufs=1) as pc_, \
                     tc.tile_pool(name="pconv_ps", bufs=2, space="PSUM") as pc_ps:
                    xw_sb = pc_.tile([P, KB * (TPC + 2)], bf16, tag="xw")
                    nc.scalar.dma_start(out=xw_sb[:], in_=xw_d[:])
                    hx_sb = pc_.tile([P, KB * 3], bf16, tag="hx")
                    nc.scalar.dma_start(out=hx_sb[:], in_=hx_d[:])
                    nc.gpsimd.dma_start(out=x1s32[:], in_=xs32_d[:])
                    ydw = pc_.tile([P, KB * (TPC + 1)], bf16, tag="ydw")
                    for k in range(KB):
                        xb = slice(k * (TPC + 2), (k + 1) * (TPC + 2))
                        w0 = wdw_sb[:, k * 3 + 0:k * 3 + 1]
                        w1 = wdw_sb[:, k * 3 + 1:k * 3 + 2]
                        w2 = wdw_sb[:, k * 3 + 2:k * 3 + 3]
                        t1 = pc_.tile([P, TPC], f32, tag="t1")
                        nc.vector.tensor_scalar_mul(
                            t1[:], xw_sb[:, k * (TPC + 2) + 2:(k + 1) * (TPC + 2)], w2)
                        t2 = pc_.tile([P, TPC], f32, tag="t2")
                        nc.vector.scalar_tensor_tensor(
                            out=t2[:], in0=xw_sb[:, k * (TPC + 2) + 1:(k + 1) * (TPC + 2) - 1],
                            scalar=w1, in1=t1[:], op0=OP.mult, op1=OP.add)
                        nc.vector.scalar_tensor_tensor(
                            out=ydw[:, k * (TPC + 1) + 1:(k + 1) * (TPC + 1)],
                            in0=xw_sb[:, k * (TPC + 2):(k + 1) * (TPC + 2) - 2],
                            scalar=w0, in1=t2[:], op0=OP.mult, op1=OP.add)
                        # prev column (token s-1) from the 3-token mini window
                        p1 = pc_.tile([P, 1], f32, tag="p1")
                        nc.vector.tensor_scalar_mul(
                            p1[:], hx_sb[:, k * 3 + 2:k * 3 + 3], w2)
                        p2 = pc_.tile([P, 1], f32, tag="p2")
                        nc.vector.scalar_tensor_tensor(
                            out=p2[:], in0=hx_sb[:, k * 3 + 1:k * 3 + 2],
                            scalar=w1, in1=p1[:], op0=OP.mult, op1=OP.add)
                        nc.vector.scalar_tensor_tensor(
                            out=ydw[:, k * (TPC + 1):k * (TPC + 1) + 1],
                            in0=hx_sb[:, k * 3:k * 3 + 1],
                            scalar=w0, in1=p2[:], op0=OP.mult, op1=OP.add)
                    for ot in range(8):
                        ps = pc_ps.tile([P, TPC + 1], f32, tag="mm")
                        for k in range(KB):
                            nc.tensor.matmul(
                                ps[:],
                                lhsT=wpw_sb[:, (k * 8 + ot) * P:(k * 8 + ot + 1) * P],
                                rhs=ydw[:, k * (TPC + 1):(k + 1) * (TPC + 1)],
                                start=(k == 0), stop=(k == KB - 1))
                        sl = slice(ot * TPC, (ot + 1) * TPC)
                        nc.vector.tensor_add(x1s32[:, sl], x1s32[:, sl], ps[:, 1:TPC + 1])
                        nc.scalar.activation(
                            x1bf[:, ot * (TPC + 1) + 1:(ot + 1) * (TPC + 1)],
                            x1s32[:, sl], AF.Copy)
                        nc.vector.tensor_add(
                            x1bf[:, ot * (TPC + 1):ot * (TPC + 1) + 1],
                            hx_sb[:, ot * 3 + 2:ot * 3 + 3], ps[:, 0:1])

                if limit < 2:
                    raise _PhaseStop

                actA = open_actA2()
                qphi = actA.tile([P, KB * TPC], bf16, tag="qphi")
                kphi = actA.tile([P, KB * TPC], bf16, tag="kphi")
                vtok = [actA.tile([P, P], bf16, tag=f"vt{i}_{c}", name=f"vt{i}_{c}")
                        for i in range(8) for c in range(2)]
                S_loc_bf = actA.tile([P, 8 * 65], bf16, tag="S_loc_bf")
                S_c0 = actA.tile([P, 8 * 65], f32, tag="S_c0")
                S_pref = actA.tile([P, 8 * 65], f32, tag="S_pref")
                S_pref_bf = actA.tile([P, 8 * 65], bf16, tag="S_pref_bf")
                S_pref1_bf = actA.tile([P, 8 * 65], bf16, tag="S_pref1_bf")
                attn_dm = actA.tile([P, KB * TPC], bf16, tag="attn_dm")

                def vt(i, c):
                    return vtok[i * 2 + c]

                # ========== Phase 2: qkv + rope + phi ==========
                def rope_pass(which, dst):
                    with tc.tile_pool(name=f"pqkv{which}", bufs=2) as pq, \
                         tc.tile_pool(name=f"pqkv{which}_ps", bufs=3,
                                      space="PSUM") as pq_ps:
                        for ot in range(8):
                            sl = slice(ot * TPC, (ot + 1) * TPC)
                            ps = pq_ps.tile([P, TPC], f32, tag="mm")
                            for k in range(KB):
                                cidx = (k * 24 + which * 8 + ot) * P
                                nc.tensor.matmul(
                                    ps[:], lhsT=wqkv_sb[:, cidx:cidx + P],
                                    rhs=x1bf[:, k * (TPC + 1) + 1:(k + 1) * (TPC + 1)],
                                    start=(k == 0), stop=(k == KB - 1))
                            qc = pq.tile([P, TPC], bf16, tag="qc")
                            nc.scalar.activation(qc[:], ps[:], AF.Copy)
                            ps2 = pq_ps.tile([P, TPC], f32, tag="mm")
                            nc.tensor.matmul(ps2[:], lhsT=p64_sb[:], rhs=qc[:],
                                             start=True, stop=True)
                            qsw = pq.tile([P, TPC], bf16, tag="qsw")
                            nc.vector.tensor_copy(out=qsw[:], in_=ps2[:])
                            t1 = pq.tile([P, TPC], bf16, tag="t1")
                            nc.vector.tensor_mul(t1[:], qc[:], ctab_sb[:])
                            t2 = pq.tile([P, TPC], bf16, tag="t2")
                            nc.vector.tensor_mul(t2[:], qsw[:], stab_sb[:])
                            qr = pq.tile([P, TPC], bf16, tag="qr")
                            nc.vector.tensor_add(qr[:], t1[:], t2[:])
                            ex = pq.tile([P, TPC], bf16, tag="ex")
                            nc.scalar.activation(ex[:], qr[:], AF.Exp)
                            rl = pq.tile([P, TPC], bf16, tag="rl")
                            nc.vector.tensor_scalar_max(rl[:], qr[:], 0.0)
                            nc.vector.scalar_tensor_tensor(
                                out=dst[:, sl], in0=ex[:], scalar=1.0, in1=rl[:],
                                op0=OP.min, op1=OP.add)

                rope_pass(1, kphi)

                if limit < 3:
                    raise _PhaseStop
                # gate-logit buffer outlives the attention pools
                gpool = contextlib.ExitStack()
                _open.append(gpool)
                pg = gpool.enter_context(tc.tile_pool(name="pg", bufs=1))
                g_sb = pg.tile([P, KB * (TPC + 1)], bf16, tag="g_sb")

                # ========== Phase 3a: transposes + local KV states + AG ==========
                with tc.tile_pool(name="pkt", bufs=3) as pkt:
                    with tc.tile_pool(name="pkv", bufs=1, space="PSUM") as pkv, \
                         tc.tile_pool(name="ptp", bufs=2, space="PSUM") as ptp, \
                         tc.tile_pool(name="pvps", bufs=2, space="PSUM") as pvps:
                        kv_ps = [pkv.tile([P, 260], f32, tag=f"kv{g}", name=f"kv{g}")
                                 for g in range(2)]
                        for g in range(2):
                            nc.vector.memset(kv_ps[g][:], 0.0)

                        def kvs(i):
                            return kv_ps[i // 4][:, (i % 4) * 65:(i % 4) * 65 + 65]

                        for i in range(8):
                            for c in range(2):
                                sl = slice(i * TPC + c * CH, i * TPC + (c + 1) * CH)
                                tp1 = ptp.tile([P, P], bf16, tag="tp")
                                nc.tensor.transpose(tp1[:], kphi[:, sl], ident_sb[:])
                                ktok = pkt.tile([P, P], bf16, tag="ktok")
                                nc.scalar.activation(ktok[:], tp1[:], AF.Copy)
                                # v for this (block, chunk) directly in
                                # [token, vdim] orientation
                                vps = pvps.tile([P, P], f32, tag="vps")
                                for k in range(KB):
                                    cidx = (k * 24 + 16 + i) * P
                                    nc.tensor.matmul(
                                        vps[:],
                                        lhsT=x1bf[:, k * (TPC + 1) + 1 + c * CH:
                                                  k * (TPC + 1) + 1 + (c + 1) * CH],
                                        rhs=wqkv_sb[:, cidx:cidx + P],
                                        start=(k == 0), stop=(k == KB - 1))
                                nc.scalar.activation(vt(i, c)[:], vps[:], AF.Copy)
                                kt = kvs(i)
                                nc.tensor.matmul(
                                    kt[0:64, 0:64], lhsT=ktok[:, 0:64],
                                    rhs=vt(i, c)[:, 0:64], start=False,
                                    stop=(c == 1), skip_group_check=True)
                                nc.tensor.matmul(
                                    kt[64:128, 0:64], lhsT=ktok[:, 64:128],
                                    rhs=vt(i, c)[:, 64:128], start=False,
                                    stop=(c == 1), tile_position=(0, 64),
                                    skip_group_check=True)
                                nc.tensor.matmul(
                                    kt[:, 64:65], lhsT=ktok[:],
                                    rhs=ones128[:], start=False,
                                    stop=(c == 1), skip_group_check=True)
                                if c == 0:
                                    nc.scalar.activation(
                                        S_c0[:, i * 65:(i + 1) * 65],
                                        kt[:], AF.Copy)
                        for i in range(8):
                            nc.scalar.activation(
                                S_loc_bf[:, i * 65:(i + 1) * 65], kvs(i)[:],
                                AF.Copy)
                    nc.gpsimd.dma_start(out=st_in[:], in_=S_loc_bf[:])
                    nc.gpsimd.collective_compute(
                        "AllGather", mybir.AluOpType.bypass, replica_groups=rg,
                        ins=[st_in[:]], outs=[st_out[:]])

                    # q projection + rope overlap the AG flight
                    rope_pass(0, qphi)

                    # ========== G matmuls (overlap AG) ==========
                    with tc.tile_pool(name="pg_ps", bufs=2, space="PSUM") as pg_ps:
                        for ot in range(8):
                            ps = pg_ps.tile([P, TPC + 1], f32, tag="mm")
                            for k in range(KB):
                                nc.tensor.matmul(
                                    ps[:],
                                    lhsT=wg_sb[:, (k * 8 + ot) * P:(k * 8 + ot + 1) * P],
                                    rhs=x1bf[:, k * (TPC + 1):(k + 1) * (TPC + 1)],
                                    start=(k == 0), stop=(k == KB - 1))
                            nc.scalar.activation(
                                g_sb[:, ot * (TPC + 1):(ot + 1) * (TPC + 1)],
                                ps[:], AF.Copy)

                    # ========== Phase 3b-e: per-chunk intra + prefix ==========
                    # chunk 0 intra overlaps the AG; readback+masked-prefix after
                    pref_done = [False]
                    with tc.tile_pool(name="patt", bufs=2, space="PSUM") as patt, \
                         tc.tile_pool(name="pstm", bufs=4) as pstm, \
                         tc.tile_pool(name="pfin", bufs=4) as pfin:
                        for c in range(2):
                            with tc.tile_pool(name=f"pnm{c}", bufs=1,
                                              space="PSUM") as pnm:
                                nmv = [pnm.tile([P, 512], f32, tag=f"nmv{g}",
                                                name=f"nmv{g}") for g in range(2)]
                                nmd = pnm.tile([P, 16], f32, tag="nmd")

                                def nv(i, h):
                                    return nmv[i // 4][:, ((i % 4) * 2 + h) * 64:
                                                       ((i % 4) * 2 + h + 1) * 64]

                                for i in range(8):
                                    for h in range(2):
                                        hs = slice(64 * h, 64 * h + 64)
                                        sl = slice(i * TPC + c * CH,
                                                   i * TPC + (c + 1) * CH)
                                        st_ps = patt.tile([P, P], f32, tag="st")
                                        nc.tensor.matmul(
                                            st_ps[:], lhsT=kphi[hs, sl],
                                            rhs=qphi[hs, sl], start=True, stop=True)
                                        stm = pstm.tile([P, P], bf16, tag="stm")
                                        nc.vector.tensor_mul(stm[:], st_ps[:],
                                                             mask_sb[:])
                                        nc.tensor.matmul(
                                            nv(i, h), lhsT=stm[:],
                                            rhs=vt(i, c)[:, hs],
                                            start=(i % 4 == 0 and h == 0),
                                            stop=False, skip_group_check=True)
                                        nc.tensor.matmul(
                                            nmd[:, i * 2 + h:i * 2 + h + 1],
                                            lhsT=stm[:], rhs=ones128[:],
                                            start=(i == 0 and h == 0), stop=False,
                                            skip_group_check=True)

                                if not pref_done[0]:
                                    pref_done[0] = True
                                    with tc.tile_pool(name="prb", bufs=3) as prb:
                                        for j in range(NC_N):
                                            gsb = prb.tile([P, 8 * 65], bf16,
                                                           tag="g")
                                            nc.sync.dma_start(
                                                out=gsb[:],
                                                in_=st_out[j * P:(j + 1) * P, :])
                                            if j == 0:
                                                nc.vector.tensor_scalar_mul(
                                                    S_pref[:], gsb[:],
                                                    mask8_sb[:, 0:1])
                                            else:
                                                nc.vector.scalar_tensor_tensor(
                                                    out=S_pref[:], in0=gsb[:],
                                                    scalar=mask8_sb[:, j:j + 1],
                                                    in1=S_pref[:],
                                                    op0=OP.mult, op1=OP.add)
                                        nc.scalar.activation(S_pref_bf[:],
                                                             S_pref[:], AF.Copy)
                                        nc.vector.tensor_add(
                                            S_pref1_bf[:], S_pref[:], S_c0[:])

                                for i in range(8):
                                    atok = pfin.tile([P, P], bf16, tag=f"atok{i % 2}")
                                    for h in range(2):
                                        hs = slice(64 * h, 64 * h + 64)
                                        sl = slice(i * TPC + c * CH,
                                                   i * TPC + (c + 1) * CH)
                                        Sb = S_pref_bf if c == 0 else S_pref1_bf
                                        nc.tensor.matmul(
                                            nv(i, h), lhsT=qphi[hs, sl],
                                            rhs=Sb[hs, i * 65:i * 65 + 64],
                                            start=False, stop=True,
                                            skip_group_check=True)
                                        nc.tensor.matmul(
                                            nmd[:, i * 2 + h:i * 2 + h + 1],
                                            lhsT=qphi[hs, sl],
                                            rhs=Sb[hs, i * 65 + 64:i * 65 + 65],
                                            start=False, stop=True,
                                            skip_group_check=True)
                                        den = pfin.tile([P, 1], f32, tag="den")
                                        nc.vector.tensor_scalar_add(
                                            den[:], nmd[:, i * 2 + h:i * 2 + h + 1],
                                            1e-6)
                                        nc.vector.reciprocal(den[:], den[:])
                                        nc.vector.tensor_scalar_mul(
                                            atok[:, 64 * h:64 * h + 64],
                                            nv(i, h), den[:])
                                    tp3 = patt.tile([P, P], bf16, tag="tp3")
                                    nc.tensor.transpose(tp3[:], atok[:], ident_sb[:])
                                    nc.scalar.activation(
                                        attn_dm[:, i * TPC + c * CH:
                                                i * TPC + (c + 1) * CH],
                                        tp3[:], AF.Copy)

                if limit < 4:
                    raise _PhaseStop
                # ========== Phase 4: wo + delta gate + x2 ==========
                with tc.tile_pool(name="pwo", bufs=2) as pw, \
                     tc.tile_pool(name="pwo_ps", bufs=2, space="PSUM") as pw_ps:
                    for ot in range(8):
                        sl = slice(ot * TPC, (ot + 1) * TPC)
                        ps = pw_ps.tile([P, TPC], f32, tag="mm")
                        for k in range(KB):
                            nc.tensor.matmul(
                                ps[:],
                                lhsT=wo_sb[:, (k * 8 + ot) * P:(k * 8 + ot + 1) * P],
                                rhs=attn_dm[:, k * TPC:(k + 1) * TPC],
                                start=(k == 0), stop=(k == KB - 1))
                        gl = pw.tile([P, TPC], bf16, tag="gl")
                        nc.vector.tensor_sub(
                            gl[:],
                            g_sb[:, ot * (TPC + 1) + 1:(ot + 1) * (TPC + 1)],
                            g_sb[:, ot * (TPC + 1):(ot + 1) * (TPC + 1) - 1])
                        gate = pw.tile([P, TPC], f32, tag="gate")
                        nc.scalar.activation(gate[:], gl[:], AF.Sigmoid,
                                             bias=bg_sb[:, ot:ot + 1])
                        ga = pw.tile([P, TPC], f32, tag="ga")
                        nc.vector.tensor_mul(ga[:], gate[:], ps[:])
                        nc.vector.tensor_add(x2s32[:, sl], x1s32[:, sl], ga[:])
                gpool.close()
                _open.remove(gpool)
                stA.close()
                _open.remove(stA)
                stW.close()
                _open.remove(stW)

                if limit < 5:
                    raise _PhaseStop
                # prefetch ALL of expert 0 (8MB) under phases 4.5-5
                stP = contextlib.ExitStack()
                _open.append(stP)
                wpre = stP.enter_context(tc.tile_pool(name="wpre", bufs=1))
                w1pre = []
                w2pre = []
                for hb in range(4):
                    t = wpre.tile([P, 4 * 2048], f8, tag=f"w1p{hb}",
                                  name=f"w1p{hb}")
                    nc.scalar.dma_start(
                        out=t[:], in_=w1s_d[:, hb * 8192:(hb + 1) * 8192])
                    w1pre.append(t)
                    t = wpre.tile([P, 4 * 2048], f8, tag=f"w2p{hb}",
                                  name=f"w2p{hb}")
                    nc.sync.dma_start(
                        out=t[:], in_=w2s_d[:, hb * 8192:(hb + 1) * 8192])
                    w2pre.append(t)
                # ========== Phase 5: LayerNorm (local) + MoE gate ==========
                with tc.tile_pool(name="pln", bufs=2) as pl, \
                     tc.tile_pool(name="pln1", bufs=1) as pl1, \
                     tc.tile_pool(name="pln_ps", bufs=1, space="PSUM") as pl_ps, \
                     tc.tile_pool(name="pln_ps2", bufs=1, space="PSUM") as pl_ps2:
                    x2bf = pl1.tile([P, KB * TPC], bf16, tag="x2bf")
                    nc.scalar.activation(x2bf[:], x2s32[:], AF.Copy)
                    x2sq = pl1.tile([P, KB * TPC], bf16, tag="x2sq")
                    nc.scalar.activation(x2sq[:], x2bf[:], AF.Square)
                    s1 = pl_ps.tile([1, TPC], f32, tag="s1")
                    s2 = pl_ps.tile([1, TPC], f32, tag="s2")
                    for k in range(KB):
                        nc.tensor.matmul(s1[:], lhsT=ones128[:],
                                         rhs=x2bf[:, k * TPC:(k + 1) * TPC],
                                         start=(k == 0), stop=(k == KB - 1))
                    for k in range(KB):
                        nc.tensor.matmul(s2[:], lhsT=ones128[:],
                                         rhs=x2sq[:, k * TPC:(k + 1) * TPC],
                                         start=(k == 0), stop=(k == KB - 1))
                    mu2 = pl1.tile([1, TPC], f32, tag="mu2")
                    nc.scalar.activation(mu2[:], s1[:], AF.Square, scale=1.0 / D)
                    var = pl1.tile([1, TPC], f32, tag="var")
                    nc.vector.scalar_tensor_tensor(
                        out=var[:], in0=s2[:], scalar=1.0 / D, in1=mu2[:],
                        op0=OP.mult, op1=OP.subtract)
                    sd = pl1.tile([1, TPC], f32, tag="sd")
                    nc.scalar.activation(sd[:], var[:], AF.Sqrt, bias=eps1[:])
                    rstd = pl1.tile([1, TPC], f32, tag="rstd")
                    nc.vector.reciprocal(rstd[:], sd[:])
                    s1r = pl1.tile([1, TPC], f32, tag="s1r")
                    nc.scalar.activation(s1r[:], s1[:], AF.Copy)
                    mu_b = pl_ps.tile([P, TPC], f32, tag="mu_b")
                    nc.tensor.matmul(mu_b[:], lhsT=ones1f[:], rhs=s1r[:],
                                     start=True, stop=True)
                    rstd_b = pl_ps.tile([P, TPC], f32, tag="rstd_b")
                    nc.tensor.matmul(rstd_b[:], lhsT=ones1f[:], rhs=rstd[:],
                                     start=True, stop=True)
                    for k in range(KB):
                        sl = slice(k * TPC, (k + 1) * TPC)
                        hp = pl.tile([P, TPC], f32, tag="hp")
                        nc.vector.scalar_tensor_tensor(
                            out=hp[:], in0=mu_b[:], scalar=-1.0 / D,
                            in1=x2s32[:, sl], op0=OP.mult, op1=OP.add)
                        h2 = pl.tile([P, TPC], f32, tag="h2")
                        nc.vector.tensor_mul(h2[:], hp[:], rstd_b[:])
                        nc.vector.tensor_scalar(
                            out=hbf[:, sl], in0=h2[:],
                            scalar1=lng_sb[:, k:k + 1], scalar2=lnb_sb[:, k:k + 1],
                            op0=OP.mult, op1=OP.add)
                    nc.scalar.activation(hf8[:], hbf[:], AF.Copy)
                    # MoE gate: softmax over 4 experts
                    lg = pl_ps2.tile([4, TPC], f32, tag="lg")
                    for k in range(KB):
                        nc.tensor.matmul(lg[:], lhsT=wmg_sb[:, k * 4:(k + 1) * 4],
                                         rhs=hbf[:, k * TPC:(k + 1) * TPC],
                                         start=(k == 0), stop=(k == KB - 1))
                    gx = pl1.tile([4, TPC], bf16, tag="gx")
                    nc.scalar.activation(gx[:], lg[:], AF.Exp)
                    sm = pl_ps2.tile([1, TPC], f32, tag="sm")
                    nc.tensor.matmul(sm[:], lhsT=ones4[:], rhs=gx[:],
                                     start=True, stop=True)
                    rc = pl1.tile([1, TPC], f32, tag="rc")
                    nc.vector.reciprocal(rc[:], sm[:])
                    for e in range(E):
                        sel = pl_ps2.tile([1, TPC], f32, tag="sel")
                        nc.tensor.matmul(sel[:], lhsT=ident_sb[0:4, e:e + 1],
                                         rhs=gx[:], start=True, stop=True)
                        gwr = pl1.tile([1, TPC], f32, tag=f"gwr{e}")
                        nc.vector.tensor_mul(gwr[:], sel[:], rc[:])
                        # fold the fp8 W2 scale (/128)
                        nc.vector.tensor_scalar_mul(gwr[:], gwr[:], 1.0 / 128.0)
                        gb_ps = pl_ps2.tile([P, TPC], f32, tag="gb")
                        nc.tensor.matmul(gb_ps[:], lhsT=ones1f[:], rhs=gwr[:],
                                         start=True, stop=True)
                        nc.scalar.activation(gwb[e][:], gb_ps[:], AF.Copy)

                if limit < 6:
                    raise _PhaseStop
                # ========== Phase 6: dense MoE, weights streamed ==========
                with tc.tile_pool(name="pmo", bufs=3) as pm, \
                     tc.tile_pool(name="phid", bufs=2) as phid, \
                     tc.tile_pool(name="pacc", bufs=1) as pacc, \
                     tc.tile_pool(name="pout_ps", bufs=1, space="PSUM") as pout_ps, \
                     tc.tile_pool(name="ppre_ps", bufs=4, space="PSUM") as ppre_ps:
                    acc = pacc.tile([P, KB * TPC], f32, tag="acc")
                    outp_t = [pout_ps.tile([P, 512], f32, tag=f"out{g}", name=f"out{g}")
                              for g in range(4)]
                    outp = [outp_t[ot // 2][:, (ot % 2) * TPC:(ot % 2 + 1) * TPC]
                            for ot in range(8)]
                    import concourse.bass as bass
                    DR = mybir.MatmulPerfMode.DoubleRow

                    def pair_ap(tile_ap, col0, inner, n=2):
                        return bass.AP(tensor=tile_ap.tensor,
                                       offset=tile_ap.offset + col0,
                                       ap=[list(tile_ap.ap[0]),
                                           [inner, n], [1, inner]])

                    for e in range(E):
                        for hb in range(4):
                            hidw = phid.tile([P, 8 * TPC], f8, tag="hidw")
                            # stream W1 for hidden block hb (1MB: 4 subs)
                            if e == 0:
                                w1t = w1pre[hb]
                            else:
                                w1t = stream.tile([P, 4 * 2048], f8, tag="w1t")
                                nc.scalar.dma_start(
                                    out=w1t[:],
                                    in_=w1s_d[:, (e * 4 + hb) * 8192:
                                              (e * 4 + hb + 1) * 8192])
                            for g in range(4):
                                s = hb * 4 + g
                                pre_t = ppre_ps.tile([P, 512], f32, tag="pre")
                                for j in range(2):
                                    ht = s * 2 + j          # hidden tile 0..31
                                    pj = g * 2 + j
                                    ps = pre_t[:, j * TPC:(j + 1) * TPC]
                                    for kp in range(4):
                                        lhsT = pair_ap(
                                            w1t[:],
                                            g * 2048 + ((kp * 2 + j) * 2) * P, P)
                                        rhs = pair_ap(hf8[:], kp * 2 * TPC, TPC)
                                        nc.tensor.matmul(
                                            ps, lhsT=lhsT, rhs=rhs,
                                            perf_mode=DR,
                                            start=(kp == 0 and j == 0),
                                            stop=(kp == 3))
                                    nc.scalar.activation(
                                        hidw[:, pj * TPC:(pj + 1) * TPC],
                                        ps, AF.Silu,
                                        bias=b1_sb[:, e * 32 + ht:e * 32 + ht + 1],
                                        scale=1.0 / 128.0)
                            # stream W2 consuming this hidden block (1MB: 4 subs)
                            if e == 0:
                                w2t = w2pre[hb]
                            else:
                                w2t = stream.tile([P, 4 * 2048], f8, tag="w2t")
                                nc.sync.dma_start(
                                    out=w2t[:],
                                    in_=w2s_d[:, (e * 4 + hb) * 8192:
                                              (e * 4 + hb + 1) * 8192])
                            for g in range(4):
                                s2 = hb * 4 + g
                                pj0 = g * 2
                                for ot in range(8):
                                    lhsT = pair_ap(w2t[:],
                                                   g * 2048 + (ot * 2) * P, P)
                                    rhs = pair_ap(hidw[:], pj0 * TPC, TPC)
                                    nc.tensor.matmul(
                                        outp[ot], lhsT=lhsT, rhs=rhs,
                                        perf_mode=DR,
                                        start=(hb == 0 and g == 0
                                               and ot % 2 == 0),
                                        stop=(hb == 3 and g == 3),
                                        skip_group_check=True)
                        # consume expert e: acc += (outp + 128*b2_e) * gw_e/128
                        for ot in range(8):
                            sl = slice(ot * TPC, (ot + 1) * TPC)
                            b2c = b2x_sb[:, e * 8 + ot:e * 8 + ot + 1]
                            if e == 0:
                                nc.vector.scalar_tensor_tensor(
                                    out=acc[:, sl], in0=outp[ot], scalar=b2c,
                                    in1=gwb[e][:], op0=OP.add, op1=OP.mult)
                            else:
                                tmp = pm.tile([P, TPC], f32, tag="tmp")
                                nc.vector.scalar_tensor_tensor(
                                    out=tmp[:], in0=outp[ot], scalar=b2c,
                                    in1=gwb[e][:], op0=OP.add, op1=OP.mult)
                                if e < E - 1:
                                    nc.vector.tensor_add(acc[:, sl],
                                                         acc[:, sl], tmp[:])
                                else:
                                    yt = pm.tile([P, TPC], f32, tag="yt")
                                    nc.vector.tensor_add(yt[:], acc[:, sl],
                                                         tmp[:])
                                    nc.vector.tensor_add(yt[:], yt[:],
                                                         x2s32[:, sl])
                                    nc.sync.dma_start(out=y_d[:, sl],
                                                      in_=yt[:])
                stP.close()
                _open.remove(stP)
                stB.close()
                _open.remove(stB)

            for _rep in range(repeat):
                try:
                    rep_body()
                except _PhaseStop:
                    for es in reversed(_open):
                        es.close()
                    _open.clear()
    nc.finalize()
    return nc


def _prep_inputs(inputs):
    x = np.asarray(inputs["x"])[0]          # [T, D] f32
    w_dw = np.asarray(inputs["w_dw"])
    w_pw = np.asarray(inputs["w_pw"])
    w_qkv = np.asarray(inputs["w_qkv"])
    w_o = np.asarray(inputs["w_o"])
    w_gate = np.asarray(inputs["w_gate"])
    b_gate = np.asarray(inputs["b_gate"])
    ln_g = np.asarray(inputs["ln_g"])
    ln_b = np.asarray(inputs["ln_b"])
    w_mg = np.asarray(inputs["w_moe_gate"])
    w1 = np.asarray(inputs["w1"])
    b1 = np.asarray(inputs["b1"])
    w2 = np.asarray(inputs["w2"])
    b2 = np.asarray(inputs["b2"])

    xT = np.ascontiguousarray(x.T).astype(np.float32)     # [D, T]
    xTp = np.pad(xT, ((0, 0), (3, 0)))                    # col t+3 = token t

    # replicated weight packs
    def pack_ko(mT):
        """[1024 in, 1024 out] -> [128, (k*8+ot)*128] lhsT blocks."""
        out = np.empty((P, 64 * P), dtype=BF)
        for k in range(KB):
            for ot in range(8):
                out[:, (k * 8 + ot) * P:(k * 8 + ot + 1) * P] = \
                    mT[k * P:(k + 1) * P, ot * P:(ot + 1) * P]
        return out

    wpw = pack_ko(w_pw.T)
    wo = pack_ko(w_o.T)
    wg = pack_ko(w_gate.T)

    perm = np.concatenate([np.arange(0, DH, 2), np.arange(1, DH, 2)])
    rows = np.arange(3 * D)
    qk_perm = rows.copy()
    for h in range(H):
        qk_perm[h * DH:(h + 1) * DH] = h * DH + perm                  # q rows
        qk_perm[D + h * DH:D + (h + 1) * DH] = D + h * DH + perm      # k rows
    wqkvT = w_qkv[qk_perm].T                                          # [1024, 3072]
    wqkv = np.empty((P, KB * 24 * P), dtype=BF)
    for k in range(KB):
        for ot in range(24):
            wqkv[:, (k * 24 + ot) * P:(k * 24 + ot + 1) * P] = \
                wqkvT[k * P:(k + 1) * P, ot * P:(ot + 1) * P]

    inv_freq = 1.0 / (10000.0 ** (np.arange(0, DH, 2, dtype=np.float32) / DH))
    pos = np.arange(T, dtype=np.float32)
    theta = pos[None, :] * inv_freq[:, None]              # [32, T]
    cos64 = np.concatenate([np.cos(theta), np.cos(theta)], axis=0)
    sin64 = np.concatenate([-np.sin(theta), np.sin(theta)], axis=0)
    ctab_full = np.tile(cos64, (2, 1)).astype(BF)         # [128, T]
    stab_full = np.tile(sin64, (2, 1)).astype(BF)

    p64 = np.zeros((P, P), dtype=BF)
    for r in range(P):
        p64[r, (r % 64 + 32) % 64 + 64 * (r // 64)] = 1.0
    mask = np.triu(np.ones((P, P), np.float32))

    wmg = np.empty((P, KB * 4), dtype=BF)
    for k in range(KB):
        wmg[:, k * 4:(k + 1) * 4] = w_mg.T[k * P:(k + 1) * P]

    b1p = np.empty((P, E * 32), dtype=np.float32)
    for e in range(E):
        b1p[:, e * 32:(e + 1) * 32] = b1[e].reshape(32, P).T
    b2xp = np.empty((P, E * 8), dtype=np.float32)
    for e in range(E):
        b2xp[:, e * 8:(e + 1) * 8] = b2[e].reshape(8, P).T * 128.0

    import ml_dtypes as _mld
    F8 = _mld.float8_e4m3

    def q8(a):
        return np.clip(a * 128.0, -240.0, 240.0).astype(F8)

    w1s = np.empty((P, E * 16 * 2048), dtype=F8)
    w2s = np.empty((P, E * 16 * 2048), dtype=F8)
    for e in range(E):
        w1eT = q8(w1[e].T)       # [1024 in, 4096 hid], x128 in fp8
        w2eT = q8(w2[e].T)       # [4096 hid, 1024 out]
        for s in range(16):
            base = (e * 16 + s) * 2048
            for kp in range(4):
                for j in range(2):
                    ht = s * 2 + j
                    for kk in range(2):
                        k = kp * 2 + kk
                        col = base + ((kp * 2 + j) * 2 + kk) * P
                        w1s[:, col:col + P] = \
                            w1eT[k * P:(k + 1) * P, ht * P:(ht + 1) * P]
            for ot in range(8):
                for cc in range(2):
                    ckg = s * 2 + cc
                    col = base + (ot * 2 + cc) * P
                    w2s[:, col:col + P] = \
                        w2eT[ckg * P:(ckg + 1) * P, ot * P:(ot + 1) * P]

    bgp = b_gate.reshape(8, P).T.astype(np.float32)
    lngp = ln_g.reshape(8, P).T.astype(np.float32)
    lnbp = ln_b.reshape(8, P).T.astype(np.float32)
    wdw = np.empty((P, KB * 3), dtype=np.float32)
    for k in range(KB):
        wdw[:, k * 3:(k + 1) * 3] = w_dw[k * P:(k + 1) * P]

    in_maps = []
    for c in range(NC_N):
        s = c * TPC
        xw = np.empty((P, KB * (TPC + 2)), dtype=BF)
        hx = np.zeros((P, KB * 3), dtype=BF)
        xs32 = np.empty((P, KB * TPC), dtype=np.float32)
        for k in range(KB):
            rb = slice(k * P, (k + 1) * P)
            # window tokens s-2 .. s+255  ->  xTp cols s+1 .. s+259
            xw[:, k * (TPC + 2):(k + 1) * (TPC + 2)] = xTp[rb, s + 1:s + TPC + 3]
            xs32[:, k * TPC:(k + 1) * TPC] = xT[rb, s:s + TPC]
            if c == 0:
                # want x1_prev == x1[0]: dw window (0, 0, x0)
                hx[:, k * 3 + 2] = xT[rb, 0]
            else:
                hx[:, k * 3:(k + 1) * 3] = xTp[rb, s:s + 3]
        mask8 = np.zeros((P, 8), dtype=np.float32)
        mask8[:, :c] = 1.0
        sofs = (c * P + np.arange(P, dtype=np.int32)).reshape(P, 1)
        in_maps.append({
            "sofs": sofs,
            "xw": xw, "hx": hx, "xs32": xs32, "wdw": wdw,
            "wpw": wpw, "wqkv": wqkv,
            "ctab": np.ascontiguousarray(ctab_full[:, s:s + TPC]),
            "stab": np.ascontiguousarray(stab_full[:, s:s + TPC]),
            "p64": p64, "mask": mask, "mask8": mask8,
            "wo": wo, "wg": wg, "bg": bgp, "lng": lngp, "lnb": lnbp,
            "wmg": wmg, "b1": b1p, "b2x": b2xp,
            "w1s": w1s, "w2s": w2s,
        })
    return in_maps


def kernel(**inputs) -> np.ndarray:
    global _PROGRAM
    from concourse.bass_utils import run_bass_kernel_spmd

    if _PROGRAM is None:
        _PROGRAM = _build_program()
    nc = _PROGRAM
    in_maps = _prep_inputs(inputs)
    last_err = None
    for _attempt in range(2):
        try:
            res = run_bass_kernel_spmd(nc, in_maps, list(range(NC_N)))
            break
        except Exception as exc:
            last_err = exc
    else:
        raise last_err
    outT = np.empty((D, T), dtype=np.float32)
    for c in range(NC_N):
        y = res.results[c]["y"]                       # [128, 8*256]
        for k in range(KB):
            outT[k * P:(k + 1) * P, c * TPC:(c + 1) * TPC] = \
                y[:, k * TPC:(k + 1) * TPC]
    return np.ascontiguousarray(outT.T)[None, :, :].astype(np.float32)



# revision 96
# speedup vs baseline: 1.0101x; 1.0101x over previous
"""Trainium2 Bass kernel for nn_DeltaNet_19430432047178 — T-sharded.

Strategy (8 cores, SPMD):
  - sequence-parallel: core c owns tokens [c*256, (c+1)*256)
  - conv window + 3-token mini-window via per-core host inputs (no
    collective); gate's G[t-1] column computed from the mini-window
  - full qkv/rope/phi local; chunked linear attention (2 chunks of 128)
    with cross-core KV prefix via ONE AllGather of per-core state sums
    + masked (per-core mask8 input) prefix reduction
  - LayerNorm fully local (d complete per token); MoE gate local
  - dense MoE: all-expert weights streamed from HBM (67MB/core)
    double-buffered under the matmuls; gw folded into hid; single out
    PSUM accumulation across experts; y written T-sharded
  - all matmuls bf16 with f32 PSUM accumulation
"""
import numpy as np
import ml_dtypes


class _PhaseStop(Exception):
    pass

NC_N = 8
T = 2048
TPC = 256
D = 1024
H = 16
DH = 64
E = 4
HD = 4096
P = 128
CH = 128
KB = 8

BF = ml_dtypes.bfloat16

_PROGRAM = None


def _build_program(limit=99, repeat=1):
    import concourse.mybir as mybir
    import concourse.tile as tile
    from concourse import bacc
    from concourse.masks import make_identity
    import contextlib

    f32 = mybir.dt.float32
    bf16 = mybir.dt.bfloat16
    f8 = mybir.dt.float8e4
    AF = mybir.ActivationFunctionType
    OP = mybir.AluOpType

    nc = bacc.Bacc()

    xw_d = nc.declare_dram_parameter("xw", [P, KB * (TPC + 2)], bf16, isOutput=False)
    hx_d = nc.declare_dram_parameter("hx", [P, KB * 3], bf16, isOutput=False)
    xs32_d = nc.declare_dram_parameter("xs32", [P, KB * TPC], f32, isOutput=False)
    wdw_d = nc.declare_dram_parameter("wdw", [P, KB * 3], f32, isOutput=False)
    wpw_d = nc.declare_dram_parameter("wpw", [P, 64 * P], bf16, isOutput=False)
    wqkv_d = nc.declare_dram_parameter("wqkv", [P, KB * 24 * P], bf16, isOutput=False)
    ctab_d = nc.declare_dram_parameter("ctab", [P, TPC], bf16, isOutput=False)
    stab_d = nc.declare_dram_parameter("stab", [P, TPC], bf16, isOutput=False)
    p64_d = nc.declare_dram_parameter("p64", [P, P], bf16, isOutput=False)
    mask_d = nc.declare_dram_parameter("mask", [P, P], f32, isOutput=False)
    mask8_d = nc.declare_dram_parameter("mask8", [P, 8], f32, isOutput=False)
    sofs_d = nc.declare_dram_parameter("sofs", [P, 1], mybir.dt.int32,
                                       isOutput=False)
    wo_d = nc.declare_dram_parameter("wo", [P, 64 * P], bf16, isOutput=False)
    wg_d = nc.declare_dram_parameter("wg", [P, 64 * P], bf16, isOutput=False)
    bg_d = nc.declare_dram_parameter("bg", [P, 8], f32, isOutput=False)
    lng_d = nc.declare_dram_parameter("lng", [P, 8], f32, isOutput=False)
    lnb_d = nc.declare_dram_parameter("lnb", [P, 8], f32, isOutput=False)
    wmg_d = nc.declare_dram_parameter("wmg", [P, KB * 4], bf16, isOutput=False)
    b1_d = nc.declare_dram_parameter("b1", [P, E * 32], f32, isOutput=False)
    b2x_d = nc.declare_dram_parameter("b2x", [P, E * 8], f32, isOutput=False)
    w1s_d = nc.declare_dram_parameter("w1s", [P, E * 8 * 4096], f8, isOutput=False)
    w2s_d = nc.declare_dram_parameter("w2s", [P, E * 8 * 4096], f8, isOutput=False)
    y_d = nc.declare_dram_parameter("y", [P, KB * TPC], f32, isOutput=True)

    rg = [list(range(NC_N))]
    st_in = nc.dram_tensor("st_in", [P, 8 * 65], bf16)
    st_out = nc.dram_tensor("st_out", [NC_N * P, 8 * 65], bf16, addr_space="Shared")

    _open = []
    with tile.TileContext(nc) as tc:
        stack = contextlib.ExitStack()
        with stack:
            consts = stack.enter_context(tc.tile_pool(name="consts", bufs=1))
            wpw_sb = consts.tile([P, 64 * P], bf16, tag="wpw")
            nc.scalar.dma_start(out=wpw_sb[:], in_=wpw_d[:])
            wqkv_sb = consts.tile([P, KB * 24 * P], bf16, tag="wqkv")
            nc.scalar.dma_start(out=wqkv_sb[:], in_=wqkv_d[:])
            wo_sb = consts.tile([P, 64 * P], bf16, tag="wo")
            nc.scalar.dma_start(out=wo_sb[:], in_=wo_d[:])
            wg_sb = consts.tile([P, 64 * P], bf16, tag="wg")
            nc.scalar.dma_start(out=wg_sb[:], in_=wg_d[:])
            wdw_sb = consts.tile([P, KB * 3], f32, tag="wdw")
            nc.scalar.dma_start(out=wdw_sb[:], in_=wdw_d[:])
            ctab_sb = consts.tile([P, TPC], bf16, tag="ctab")
            nc.scalar.dma_start(out=ctab_sb[:], in_=ctab_d[:])
            stab_sb = consts.tile([P, TPC], bf16, tag="stab")
            nc.scalar.dma_start(out=stab_sb[:], in_=stab_d[:])
            p64_sb = consts.tile([P, P], bf16, tag="p64")
            nc.scalar.dma_start(out=p64_sb[:], in_=p64_d[:])
            mask_sb = consts.tile([P, P], f32, tag="mask")
            nc.scalar.dma_start(out=mask_sb[:], in_=mask_d[:])
            mask8_sb = consts.tile([P, 8], f32, tag="mask8")
            nc.scalar.dma_start(out=mask8_sb[:], in_=mask8_d[:])
            sofs_sb = consts.tile([P, 1], mybir.dt.int32, tag="sofs")
            nc.scalar.dma_start(out=sofs_sb[:], in_=sofs_d[:])
            bg_sb = consts.tile([P, 8], f32, tag="bg")
            nc.scalar.dma_start(out=bg_sb[:], in_=bg_d[:])
            lng_sb = consts.tile([P, 8], f32, tag="lng")
            nc.scalar.dma_start(out=lng_sb[:], in_=lng_d[:])
            lnb_sb = consts.tile([P, 8], f32, tag="lnb")
            nc.scalar.dma_start(out=lnb_sb[:], in_=lnb_d[:])
            wmg_sb = consts.tile([P, KB * 4], bf16, tag="wmg")
            nc.scalar.dma_start(out=wmg_sb[:], in_=wmg_d[:])
            b1_sb = consts.tile([P, E * 32], f32, tag="b1")
            nc.scalar.dma_start(out=b1_sb[:], in_=b1_d[:])
            b2x_sb = consts.tile([P, E * 8], f32, tag="b2x")
            nc.scalar.dma_start(out=b2x_sb[:], in_=b2x_d[:])
            ident_sb = consts.tile([P, P], bf16, tag="ident")
            make_identity(nc, ident_sb[:])
            ones128 = consts.tile([P, 1], bf16, tag="ones128")
            nc.vector.memset(ones128[:], 1.0)
            ones4 = consts.tile([4, 1], bf16, tag="ones4")
            nc.vector.memset(ones4[:], 1.0)
            ones1f = consts.tile([1, P], f32, tag="ones1f")
            nc.vector.memset(ones1f[:], 1.0)
            eps1 = consts.tile([1, 1], f32, tag="eps1")
            nc.vector.memset(eps1[:], 1e-5)
            stream = stack.enter_context(tc.tile_pool(name="stream", bufs=2))

            def rep_body():
                # persistent-through-MoE activations
                stB = contextlib.ExitStack()
                _open.append(stB)
                actB = stB.enter_context(tc.tile_pool(name="actB", bufs=1))
                x2s32 = actB.tile([P, KB * TPC], f32, tag="x2s32")
                hbf = actB.tile([P, KB * TPC], bf16, tag="hbf")
                hf8 = actB.tile([P, KB * TPC], f8, tag="hf8")
                gwb = [actB.tile([P, TPC], f32, tag=f"gwb{e}", name=f"gwb{e}")
                       for e in range(E)]
                # activations dead after wo/gate
                stA = contextlib.ExitStack()
                _open.append(stA)
                actA1 = stA.enter_context(tc.tile_pool(name="actA1", bufs=1))
                x1s32 = actA1.tile([P, KB * TPC], f32, tag="x1s32")
                x1bf = actA1.tile([P, KB * (TPC + 1)], bf16, tag="x1bf")

                def open_actA2():
                    return stA.enter_context(tc.tile_pool(name="actA2", bufs=1))

                # ========== Phase 1: conv mixer ==========
                with tc.tile_pool(name="pconv", bufs=1) as pc_, \
                     tc.tile_pool(name="pconv_ps", bufs=2, space="PSUM") as pc_ps:
                    xw_sb = pc_.tile([P, KB * (TPC + 2)], bf16, tag="xw")
                    nc.scalar.dma_start(out=xw_sb[:], in_=xw_d[:])
                    hx_sb = pc_.tile([P, KB * 3], bf16, tag="hx")
                    nc.scalar.dma_start(out=hx_sb[:], in_=hx_d[:])
                    nc.gpsimd.dma_start(out=x1s32[:], in_=xs32_d[:])
                    ydw = pc_.tile([P, KB * (TPC + 1)], bf16, tag="ydw")
                    for k in range(KB):
                        xb = slice(k * (TPC + 2), (k + 1) * (TPC + 2))
                        w0 = wdw_sb[:, k * 3 + 0:k * 3 + 1]
                        w1 = wdw_sb[:, k * 3 + 1:k * 3 + 2]
                        w2 = wdw_sb[:, k * 3 + 2:k * 3 + 3]
                        t1 = pc_.tile([P, TPC], f32, tag="t1")
                        nc.vector.tensor_scalar_mul(
                            t1[:], xw_sb[:, k * (TPC + 2) + 2:(k + 1) * (TPC + 2)], w2)
                        t2 = pc_.tile([P, TPC], f32, tag="t2")
                        nc.vector.scalar_tensor_tensor(
                            out=t2[:], in0=xw_sb[:, k * (TPC + 2) + 1:(k + 1) * (TPC + 2) - 1],
                            scalar=w1, in1=t1[:], op0=OP.mult, op1=OP.add)
                        nc.vector.scalar_tensor_tensor(
                            out=ydw[:, k * (TPC + 1) + 1:(k + 1) * (TPC + 1)],
                            in0=xw_sb[:, k * (TPC + 2):(k + 1) * (TPC + 2) - 2],
                            scalar=w0, in1=t2[:], op0=OP.mult, op1=OP.add)
                        # prev column (token s-1) from the 3-token mini window
                        p1 = pc_.tile([P, 1], f32, tag="p1")
                        nc.vector.tensor_scalar_mul(
                            p1[:], hx_sb[:, k * 3 + 2:k * 3 + 3], w2)
                        p2 = pc_.tile([P, 1], f32, tag="p2")
                        nc.vector.scalar_tensor_tensor(
                            out=p2[:], in0=hx_sb[:, k * 3 + 1:k * 3 + 2],
                            scalar=w1, in1=p1[:], op0=OP.mult, op1=OP.add)
                        nc.vector.scalar_tensor_tensor(
                            out=ydw[:, k * (TPC + 1):k * (TPC + 1) + 1],
                            in0=hx_sb[:, k * 3:k * 3 + 1],
                            scalar=w0, in1=p2[:], op0=OP.mult, op1=OP.add)
                    for ot in range(8):
                        ps = pc_ps.tile([P, TPC + 1], f32, tag="mm")
                        for k in range(KB):
                            nc.tensor.matmul(
                                ps[:],
                                lhsT=wpw_sb[:, (k * 8 + ot) * P:(k * 8 + ot + 1) * P],
                                rhs=ydw[:, k * (TPC + 1):(k + 1) * (TPC + 1)],
                                start=(k == 0), stop=(k == KB - 1))
                        sl = slice(ot * TPC, (ot + 1) * TPC)
                        nc.vector.tensor_add(x1s32[:, sl], x1s32[:, sl], ps[:, 1:TPC + 1])
                        nc.scalar.activation(
                            x1bf[:, ot * (TPC + 1) + 1:(ot + 1) * (TPC + 1)],
                            x1s32[:, sl], AF.Copy)
                        nc.vector.tensor_add(
                            x1bf[:, ot * (TPC + 1):ot * (TPC + 1) + 1],
                            hx_sb[:, ot * 3 + 2:ot * 3 + 3], ps[:, 0:1])

                if limit < 2:
                    raise _PhaseStop

                actA = open_actA2()
                qphi = actA.tile([P, KB * TPC], bf16, tag="qphi")
                kphi = actA.tile([P, KB * TPC], bf16, tag="kphi")
                vtok = [actA.tile([P, P], bf16, tag=f"vt{i}_{c}", name=f"vt{i}_{c}")
                        for i in range(8) for c in range(2)]
                S_loc_bf = actA.tile([P, 8 * 65], bf16, tag="S_loc_bf")
                S_c0 = actA.tile([P, 8 * 65], f32, tag="S_c0")
                S_pref = actA.tile([P, 8 * 65], f32, tag="S_pref")
                S_pref_bf = actA.tile([P, 8 * 65], bf16, tag="S_pref_bf")
                S_pref1_bf = actA.tile([P, 8 * 65], bf16, tag="S_pref1_bf")
                attn_dm = actA.tile([P, KB * TPC], bf16, tag="attn_dm")

                def vt(i, c):
                    return vtok[i * 2 + c]

                # ========== Phase 2: qkv + rope + phi ==========
                def rope_pass(which, dst):
                    with tc.tile_pool(name=f"pqkv{which}", bufs=2) as pq, \
                         tc.tile_pool(name=f"pqkv{which}_ps", bufs=3,
                                      space="PSUM") as pq_ps:
                        for ot in range(8):
                            sl = slice(ot * TPC, (ot + 1) * TPC)
                            ps = pq_ps.tile([P, TPC], f32, tag="mm")
                            for k in range(KB):
                                cidx = (k * 24 + which * 8 + ot) * P
                                nc.tensor.matmul(
                                    ps[:], lhsT=wqkv_sb[:, cidx:cidx + P],
                                    rhs=x1bf[:, k * (TPC + 1) + 1:(k + 1) * (TPC + 1)],
                                    start=(k == 0), stop=(k == KB - 1))
                            qc = pq.tile([P, TPC], bf16, tag="qc")
                            nc.scalar.activation(qc[:], ps[:], AF.Copy)
                            ps2 = pq_ps.tile([P, TPC], f32, tag="mm")
                            nc.tensor.matmul(ps2[:], lhsT=p64_sb[:], rhs=qc[:],
                                             start=True, stop=True)
                            qsw = pq.tile([P, TPC], bf16, tag="qsw")
                            nc.vector.tensor_copy(out=qsw[:], in_=ps2[:])
                            t1 = pq.tile([P, TPC], bf16, tag="t1")
                            nc.vector.tensor_mul(t1[:], qc[:], ctab_sb[:])
                            t2 = pq.tile([P, TPC], bf16, tag="t2")
                            nc.vector.tensor_mul(t2[:], qsw[:], stab_sb[:])
                            qr = pq.tile([P, TPC], bf16, tag="qr")
                            nc.vector.tensor_add(qr[:], t1[:], t2[:])
                            ex = pq.tile([P, TPC], bf16, tag="ex")
                            nc.scalar.activation(ex[:], qr[:], AF.Exp)
                            rl = pq.tile([P, TPC], bf16, tag="rl")
                            nc.vector.tensor_scalar_max(rl[:], qr[:], 0.0)
                            nc.vector.scalar_tensor_tensor(
                                out=dst[:, sl], in0=ex[:], scalar=1.0, in1=rl[:],
                                op0=OP.min, op1=OP.add)

                rope_pass(1, kphi)

                if limit < 3:
                    raise _PhaseStop
                # gate-logit buffer outlives the attention pools
                gpool = contextlib.ExitStack()
                _open.append(gpool)
                pg = gpool.enter_context(tc.tile_pool(name="pg", bufs=1))
                g_sb = pg.tile([P, KB * (TPC + 1)], bf16, tag="g_sb")

                # ========== Phase 3a: transposes + local KV states + AG ==========
                with tc.tile_pool(name="pkt", bufs=3) as pkt:
                    with tc.tile_pool(name="pkv", bufs=1, space="PSUM") as pkv, \
                         tc.tile_pool(name="ptp", bufs=2, space="PSUM") as ptp, \
                         tc.tile_pool(name="pvps", bufs=2, space="PSUM") as pvps:
                        kv_ps = [pkv.tile([P, 260], f32, tag=f"kv{g}", name=f"kv{g}")
                                 for g in range(2)]
                        for g in range(2):
                            nc.vector.memset(kv_ps[g][:], 0.0)

                        def kvs(i):
                            return kv_ps[i // 4][:, (i % 4) * 65:(i % 4) * 65 + 65]

                        for i in range(8):
                            for c in range(2):
                                sl = slice(i * TPC + c * CH, i * TPC + (c + 1) * CH)
                                tp1 = ptp.tile([P, P], bf16, tag="tp")
                                nc.tensor.transpose(tp1[:], kphi[:, sl], ident_sb[:])
                                ktok = pkt.tile([P, P], bf16, tag="ktok")
                                nc.scalar.activation(ktok[:], tp1[:], AF.Copy)
                                # v for this (block, chunk) directly in
                                # [token, vdim] orientation
                                vps = pvps.tile([P, P], f32, tag="vps")
                                for k in range(KB):
                                    cidx = (k * 24 + 16 + i) * P
                                    nc.tensor.matmul(
                                        vps[:],
                                        lhsT=x1bf[:, k * (TPC + 1) + 1 + c * CH:
                                                  k * (TPC + 1) + 1 + (c + 1) * CH],
                                        rhs=wqkv_sb[:, cidx:cidx + P],
                                        start=(k == 0), stop=(k == KB - 1))
                                nc.scalar.activation(vt(i, c)[:], vps[:], AF.Copy)
                                kt = kvs(i)
                                nc.tensor.matmul(
                                    kt[0:64, 0:64], lhsT=ktok[:, 0:64],
                                    rhs=vt(i, c)[:, 0:64], start=False,
                                    stop=(c == 1), skip_group_check=True)
                                nc.tensor.matmul(
                                    kt[64:128, 0:64], lhsT=ktok[:, 64:128],
                                    rhs=vt(i, c)[:, 64:128], start=False,
                                    stop=(c == 1), tile_position=(0, 64),
                                    skip_group_check=True)
                                nc.tensor.matmul(
                                    kt[:, 64:65], lhsT=ktok[:],
                                    rhs=ones128[:], start=False,
                                    stop=(c == 1), skip_group_check=True)
                                if c == 0:
                                    nc.scalar.activation(
                                        S_c0[:, i * 65:(i + 1) * 65],
                                        kt[:], AF.Copy)
                        for i in range(8):
                            nc.scalar.activation(
                                S_loc_bf[:, i * 65:(i + 1) * 65], kvs(i)[:],
                                AF.Copy)
                    nc.gpsimd.dma_start(out=st_in[:], in_=S_loc_bf[:])
                    nc.gpsimd.collective_compute(
                        "AllGather", mybir.AluOpType.bypass, replica_groups=rg,
                        ins=[st_in[:]], outs=[st_out[:]])

                    # q projection + rope overlap the AG flight
                    rope_pass(0, qphi)

                    # ========== G matmuls (overlap AG) ==========
                    with tc.tile_pool(name="pg_ps", bufs=2, space="PSUM") as pg_ps:
                        for ot in range(8):
                            ps = pg_ps.tile([P, TPC + 1], f32, tag="mm")
                            for k in range(KB):
                                nc.tensor.matmul(
                                    ps[:],
                                    lhsT=wg_sb[:, (k * 8 + ot) * P:(k * 8 + ot + 1) * P],
                                    rhs=x1bf[:, k * (TPC + 1):(k + 1) * (TPC + 1)],
                                    start=(k == 0), stop=(k == KB - 1))
                            nc.scalar.activation(
                                g_sb[:, ot * (TPC + 1):(ot + 1) * (TPC + 1)],
                                ps[:], AF.Copy)

                    # ========== Phase 3b-e: per-chunk intra + prefix ==========
                    # chunk 0 intra overlaps the AG; readback+masked-prefix after
                    pref_done = [False]
                    with tc.tile_pool(name="patt", bufs=2, space="PSUM") as patt, \
                         tc.tile_pool(name="pstm", bufs=4) as pstm, \
                         tc.tile_pool(name="pfin", bufs=4) as pfin:
                        for c in range(2):
                            with tc.tile_pool(name=f"pnm{c}", bufs=1,
                                              space="PSUM") as pnm:
                                nmv = [pnm.tile([P, 512], f32, tag=f"nmv{g}",
                                                name=f"nmv{g}") for g in range(2)]
                                nmd = pnm.tile([P, 16], f32, tag="nmd")

                                def nv(i, h):
                                    return nmv[i // 4][:, ((i % 4) * 2 + h) * 64:
                                                       ((i % 4) * 2 + h + 1) * 64]

                                for i in range(8):
                                    for h in range(2):
                                        hs = slice(64 * h, 64 * h + 64)
                                        sl = slice(i * TPC + c * CH,
                                                   i * TPC + (c + 1) * CH)
                                        st_ps = patt.tile([P, P], f32, tag="st")
                                        nc.tensor.matmul(
                                            st_ps[:], lhsT=kphi[hs, sl],
                                            rhs=qphi[hs, sl], start=True, stop=True)
                                        stm = pstm.tile([P, P], bf16, tag="stm")
                                        nc.vector.tensor_mul(stm[:], st_ps[:],
                                                             mask_sb[:])
                                        nc.tensor.matmul(
                                            nv(i, h), lhsT=stm[:],
                                            rhs=vt(i, c)[:, hs],
                                            start=(i % 4 == 0 and h == 0),
                                            stop=False, skip_group_check=True)
                                        nc.tensor.matmul(
                                            nmd[:, i * 2 + h:i * 2 + h + 1],
                                            lhsT=stm[:], rhs=ones128[:],
                                            start=(i == 0 and h == 0), stop=False,
                                            skip_group_check=True)

                                if not pref_done[0]:
                                    pref_done[0] = True
                                    with tc.tile_pool(name="prb", bufs=3) as prb:
                                        for j in range(NC_N):
                                            gsb = prb.tile([P, 8 * 65], bf16,
                                                           tag="g")
                                            nc.sync.dma_start(
                                                out=gsb[:],
                                                in_=st_out[j * P:(j + 1) * P, :])
                                            if j == 0:
                                                nc.vector.tensor_scalar_mul(
                                                    S_pref[:], gsb[:],
                                                    mask8_sb[:, 0:1])
                                            else:
                                                nc.vector.scalar_tensor_tensor(
                                                    out=S_pref[:], in0=gsb[:],
                                                    scalar=mask8_sb[:, j:j + 1],
                                                    in1=S_pref[:],
                                                    op0=OP.mult, op1=OP.add)
                                        nc.scalar.activation(S_pref_bf[:],
                                                             S_pref[:], AF.Copy)
                                        nc.vector.tensor_add(
                                            S_pref1_bf[:], S_pref[:], S_c0[:])

                                for i in range(8):
                                    atok = pfin.tile([P, P], bf16, tag=f"atok{i % 2}")
                                    for h in range(2):
                                        hs = slice(64 * h, 64 * h + 64)
                                        sl = slice(i * TPC + c * CH,
                                                   i * TPC + (c + 1) * CH)
                                        Sb = S_pref_bf if c == 0 else S_pref1_bf
                                        nc.tensor.matmul(
                                            nv(i, h), lhsT=qphi[hs, sl],
                                            rhs=Sb[hs, i * 65:i * 65 + 64],
                                            start=False, stop=True,
                                            skip_group_check=True)
                                        nc.tensor.matmul(
                                            nmd[:, i * 2 + h:i * 2 + h + 1],
                                            lhsT=qphi[hs, sl],
                                            rhs=Sb[hs, i * 65 + 64:i * 65 + 65],
                                            start=False, stop=True,
                                            skip_group_check=True)
                                        den = pfin.tile([P, 1], f32, tag="den")
                                        nc.vector.tensor_scalar_add(
                                            den[:], nmd[:, i * 2 + h:i * 2 + h + 1],
                                            1e-6)
                                        nc.vector.reciprocal(den[:], den[:])
                                        nc.vector.tensor_scalar_mul(
                                            atok[:, 64 * h:64 * h + 64],
                                            nv(i, h), den[:])
                                    tp3 = patt.tile([P, P], bf16, tag="tp3")
                                    nc.tensor.transpose(tp3[:], atok[:], ident_sb[:])
                                    nc.scalar.activation(
                                        attn_dm[:, i * TPC + c * CH:
                                                i * TPC + (c + 1) * CH],
                                        tp3[:], AF.Copy)

                if limit < 4:
                    raise _PhaseStop
                # ========== Phase 4: wo + delta gate + x2 ==========
                # prefetch the first MoE weight tiles under phases 4-5
                w1pre = []
                w2pre = []
                for pi in range(2):
                    t = stream.tile([P, 4 * 2048], f8, tag="w1t",
                                    name=f"w1t_p{pi}")
                    nc.scalar.dma_start(
                        out=t[:], in_=w1s_d[:, pi * 8192:(pi + 1) * 8192])
                    w1pre.append(t)
                    t = stream.tile([P, 4 * 2048], f8, tag="w2t",
                                    name=f"w2t_p{pi}")
                    nc.sync.dma_start(
                        out=t[:], in_=w2s_d[:, pi * 8192:(pi + 1) * 8192])
                    w2pre.append(t)
                with tc.tile_pool(name="pwo", bufs=2) as pw, \
                     tc.tile_pool(name="pwo_ps", bufs=2, space="PSUM") as pw_ps:
                    for ot in range(8):
                        sl = slice(ot * TPC, (ot + 1) * TPC)
                        ps = pw_ps.tile([P, TPC], f32, tag="mm")
                        for k in range(KB):
                            nc.tensor.matmul(
                                ps[:],
                                lhsT=wo_sb[:, (k * 8 + ot) * P:(k * 8 + ot + 1) * P],
                                rhs=attn_dm[:, k * TPC:(k + 1) * TPC],
                                start=(k == 0), stop=(k == KB - 1))
                        gl = pw.tile([P, TPC], bf16, tag="gl")
                        nc.vector.tensor_sub(
                            gl[:],
                            g_sb[:, ot * (TPC + 1) + 1:(ot + 1) * (TPC + 1)],
                            g_sb[:, ot * (TPC + 1):(ot + 1) * (TPC + 1) - 1])
                        gate = pw.tile([P, TPC], f32, tag="gate")
                        nc.scalar.activation(gate[:], gl[:], AF.Sigmoid,
                                             bias=bg_sb[:, ot:ot + 1])
                        ga = pw.tile([P, TPC], f32, tag="ga")
                        nc.vector.tensor_mul(ga[:], gate[:], ps[:])
                        nc.vector.tensor_add(x2s32[:, sl], x1s32[:, sl], ga[:])
                gpool.close()
                _open.remove(gpool)
                stA.close()
                _open.remove(stA)

                if limit < 5:
                    raise _PhaseStop
                # ========== Phase 5: LayerNorm (local) + MoE gate ==========
                with tc.tile_pool(name="pln", bufs=2) as pl, \
                     tc.tile_pool(name="pln1", bufs=1) as pl1, \
                     tc.tile_pool(name="pln_ps", bufs=1, space="PSUM") as pl_ps, \
                     tc.tile_pool(name="pln_ps2", bufs=1, space="PSUM") as pl_ps2:
                    x2bf = pl1.tile([P, KB * TPC], bf16, tag="x2bf")
                    nc.scalar.activation(x2bf[:], x2s32[:], AF.Copy)
                    x2sq = pl1.tile([P, KB * TPC], bf16, tag="x2sq")
                    nc.scalar.activation(x2sq[:], x2bf[:], AF.Square)
                    s1 = pl_ps.tile([1, TPC], f32, tag="s1")
                    s2 = pl_ps.tile([1, TPC], f32, tag="s2")
                    for k in range(KB):
                        nc.tensor.matmul(s1[:], lhsT=ones128[:],
                                         rhs=x2bf[:, k * TPC:(k + 1) * TPC],
                                         start=(k == 0), stop=(k == KB - 1))
                    for k in range(KB):
                        nc.tensor.matmul(s2[:], lhsT=ones128[:],
                                         rhs=x2sq[:, k * TPC:(k + 1) * TPC],
                                         start=(k == 0), stop=(k == KB - 1))
                    mu2 = pl1.tile([1, TPC], f32, tag="mu2")
                    nc.scalar.activation(mu2[:], s1[:], AF.Square, scale=1.0 / D)
                    var = pl1.tile([1, TPC], f32, tag="var")
                    nc.vector.scalar_tensor_tensor(
                        out=var[:], in0=s2[:], scalar=1.0 / D, in1=mu2[:],
                        op0=OP.mult, op1=OP.subtract)
                    sd = pl1.tile([1, TPC], f32, tag="sd")
                    nc.scalar.activation(sd[:], var[:], AF.Sqrt, bias=eps1[:])
                    rstd = pl1.tile([1, TPC], f32, tag="rstd")
                    nc.vector.reciprocal(rstd[:], sd[:])
                    s1r = pl1.tile([1, TPC], f32, tag="s1r")
                    nc.scalar.activation(s1r[:], s1[:], AF.Copy)
                    mu_b = pl_ps.tile([P, TPC], f32, tag="mu_b")
                    nc.tensor.matmul(mu_b[:], lhsT=ones1f[:], rhs=s1r[:],
                                     start=True, stop=True)
                    rstd_b = pl_ps.tile([P, TPC], f32, tag="rstd_b")
                    nc.tensor.matmul(rstd_b[:], lhsT=ones1f[:], rhs=rstd[:],
                                     start=True, stop=True)
                    for k in range(KB):
                        sl = slice(k * TPC, (k + 1) * TPC)
                        hp = pl.tile([P, TPC], f32, tag="hp")
                        nc.vector.scalar_tensor_tensor(
                            out=hp[:], in0=mu_b[:], scalar=-1.0 / D,
                            in1=x2s32[:, sl], op0=OP.mult, op1=OP.add)
                        h2 = pl.tile([P, TPC], f32, tag="h2")
                        nc.vector.tensor_mul(h2[:], hp[:], rstd_b[:])
                        nc.vector.tensor_scalar(
                            out=hbf[:, sl], in0=h2[:],
                            scalar1=lng_sb[:, k:k + 1], scalar2=lnb_sb[:, k:k + 1],
                            op0=OP.mult, op1=OP.add)
                    nc.scalar.activation(hf8[:], hbf[:], AF.Copy)
                    # MoE gate: softmax over 4 experts
                    lg = pl_ps2.tile([4, TPC], f32, tag="lg")
                    for k in range(KB):
                        nc.tensor.matmul(lg[:], lhsT=wmg_sb[:, k * 4:(k + 1) * 4],
                                         rhs=hbf[:, k * TPC:(k + 1) * TPC],
                                         start=(k == 0), stop=(k == KB - 1))
                    gx = pl1.tile([4, TPC], bf16, tag="gx")
                    nc.scalar.activation(gx[:], lg[:], AF.Exp)
                    sm = pl_ps2.tile([1, TPC], f32, tag="sm")
                    nc.tensor.matmul(sm[:], lhsT=ones4[:], rhs=gx[:],
                                     start=True, stop=True)
                    rc = pl1.tile([1, TPC], f32, tag="rc")
                    nc.vector.reciprocal(rc[:], sm[:])
                    for e in range(E):
                        sel = pl_ps2.tile([1, TPC], f32, tag="sel")
                        nc.tensor.matmul(sel[:], lhsT=ident_sb[0:4, e:e + 1],
                                         rhs=gx[:], start=True, stop=True)
                        gwr = pl1.tile([1, TPC], f32, tag=f"gwr{e}")
                        nc.vector.tensor_mul(gwr[:], sel[:], rc[:])
                        # fold the fp8 W2 scale (/128)
                        nc.vector.tensor_scalar_mul(gwr[:], gwr[:], 1.0 / 128.0)
                        gb_ps = pl_ps2.tile([P, TPC], f32, tag="gb")
                        nc.tensor.matmul(gb_ps[:], lhsT=ones1f[:], rhs=gwr[:],
                                         start=True, stop=True)
                        nc.scalar.activation(gwb[e][:], gb_ps[:], AF.Copy)

                if limit < 6:
                    raise _PhaseStop
                # ========== Phase 6: dense MoE, weights streamed ==========
                with tc.tile_pool(name="pmo", bufs=3) as pm, \
                     tc.tile_pool(name="phid", bufs=2) as phid, \
                     tc.tile_pool(name="pacc", bufs=1) as pacc, \
                     tc.tile_pool(name="pout_ps", bufs=1, space="PSUM") as pout_ps, \
                     tc.tile_pool(name="ppre_ps", bufs=4, space="PSUM") as ppre_ps:
                    acc = pacc.tile([P, KB * TPC], f32, tag="acc")
                    outp_t = [pout_ps.tile([P, 512], f32, tag=f"out{g}", name=f"out{g}")
                              for g in range(4)]
                    outp = [outp_t[ot // 2][:, (ot % 2) * TPC:(ot % 2 + 1) * TPC]
                            for ot in range(8)]
                    import concourse.bass as bass
                    DR = mybir.MatmulPerfMode.DoubleRow

                    def pair_ap(tile_ap, col0, inner, n=2):
                        return bass.AP(tensor=tile_ap.tensor,
                                       offset=tile_ap.offset + col0,
                                       ap=[list(tile_ap.ap[0]),
                                           [inner, n], [1, inner]])

                    for e in range(E):
                        for hb in range(4):
                            hidw = phid.tile([P, 8 * TPC], f8, tag="hidw")
                            # stream W1 for hidden block hb (1MB: 4 subs)
                            if e == 0 and hb < 2:
                                w1t = w1pre[hb]
                            else:
                                w1t = stream.tile([P, 4 * 2048], f8, tag="w1t")
                                nc.scalar.dma_start(
                                    out=w1t[:],
                                    in_=w1s_d[:, (e * 4 + hb) * 8192:
                                              (e * 4 + hb + 1) * 8192])
                            for g in range(4):
                                s = hb * 4 + g
                                pre_t = ppre_ps.tile([P, 512], f32, tag="pre")
                                for j in range(2):
                                    ht = s * 2 + j          # hidden tile 0..31
                                    pj = g * 2 + j
                                    ps = pre_t[:, j * TPC:(j + 1) * TPC]
                                    for kp in range(4):
                                        lhsT = pair_ap(
                                            w1t[:],
                                            g * 2048 + ((kp * 2 + j) * 2) * P, P)
                                        rhs = pair_ap(hf8[:], kp * 2 * TPC, TPC)
                                        nc.tensor.matmul(
                                            ps, lhsT=lhsT, rhs=rhs,
                                            perf_mode=DR,
                                            start=(kp == 0 and j == 0),
                                            stop=(kp == 3))
                                    nc.scalar.activation(
                                        hidw[:, pj * TPC:(pj + 1) * TPC],
                                        ps, AF.Silu,
                                        bias=b1_sb[:, e * 32 + ht:e * 32 + ht + 1],
                                        scale=1.0 / 128.0)
                            # stream W2 consuming this hidden block (1MB: 4 subs)
                            if e == 0 and hb < 2:
                                w2t = w2pre[hb]
                            else:
                                w2t = stream.tile([P, 4 * 2048], f8, tag="w2t")
                                nc.sync.dma_start(
                                    out=w2t[:],
                                    in_=w2s_d[:, (e * 4 + hb) * 8192:
                                              (e * 4 + hb + 1) * 8192])
                            for g in range(4):
                                s2 = hb * 4 + g
                                pj0 = g * 2
                                for ot in range(8):
                                    lhsT = pair_ap(w2t[:],
                                                   g * 2048 + (ot * 2) * P, P)
                                    rhs = pair_ap(hidw[:], pj0 * TPC, TPC)
                                    nc.tensor.matmul(
                                        outp[ot], lhsT=lhsT, rhs=rhs,
                                        perf_mode=DR,
                                        start=(hb == 0 and g == 0
                                               and ot % 2 == 0),
                                        stop=(hb == 3 and g == 3),
                                        skip_group_check=True)
                        # consume expert e: acc += (outp + 128*b2_e) * gw_e/128
                        for ot in range(8):
                            sl = slice(ot * TPC, (ot + 1) * TPC)
                            b2c = b2x_sb[:, e * 8 + ot:e * 8 + ot + 1]
                            if e == 0:
                                nc.vector.scalar_tensor_tensor(
                                    out=acc[:, sl], in0=outp[ot], scalar=b2c,
                                    in1=gwb[e][:], op0=OP.add, op1=OP.mult)
                            else:
                                tmp = pm.tile([P, TPC], f32, tag="tmp")
                                nc.vector.scalar_tensor_tensor(
                                    out=tmp[:], in0=outp[ot], scalar=b2c,
                                    in1=gwb[e][:], op0=OP.add, op1=OP.mult)
                                if e < E - 1:
                                    nc.vector.tensor_add(acc[:, sl],
                                                         acc[:, sl], tmp[:])
                                else:
                                    yt = pm.tile([P, TPC], f32, tag="yt")
                                    nc.vector.tensor_add(yt[:], acc[:, sl],
                                                         tmp[:])
                                    nc.vector.tensor_add(yt[:], yt[:],
                                                         x2s32[:, sl])
                                    nc.sync.dma_start(out=y_d[:, sl],
                                                      in_=yt[:])
                stB.close()
                _open.remove(stB)

            for _rep in range(repeat):
                try:
                    rep_body()
                except _PhaseStop:
                    for es in reversed(_open):
                        es.close()
                    _open.clear()
    nc.finalize()
    return nc


def _prep_inputs(inputs):
    x = np.asarray(inputs["x"])[0]          # [T, D] f32
    w_dw = np.asarray(inputs["w_dw"])
    w_pw = np.asarray(inputs["w_pw"])
    w_qkv = np.asarray(inputs["w_qkv"])
    w_o = np.asarray(inputs["w_o"])
    w_gate = np.asarray(inputs["w_gate"])
    b_gate = np.asarray(inputs["b_gate"])
    ln_g = np.asarray(inputs["ln_g"])
    ln_b = np.asarray(inputs["ln_b"])
    w_mg = np.asarray(inputs["w_moe_gate"])
    w1 = np.asarray(inputs["w1"])
    b1 = np.asarray(inputs["b1"])
    w2 = np.asarray(inputs["w2"])
    b2 = np.asarray(inputs["b2"])

    xT = np.ascontiguousarray(x.T).astype(np.float32)     # [D, T]
    xTp = np.pad(xT, ((0, 0), (3, 0)))                    # col t+3 = token t

    # replicated weight packs
    def pack_ko(mT):
        """[1024 in, 1024 out] -> [128, (k*8+ot)*128] lhsT blocks."""
        out = np.empty((P, 64 * P), dtype=BF)
        for k in range(KB):
            for ot in range(8):
                out[:, (k * 8 + ot) * P:(k * 8 + ot + 1) * P] = \
                    mT[k * P:(k + 1) * P, ot * P:(ot + 1) * P]
        return out

    wpw = pack_ko(w_pw.T)
    wo = pack_ko(w_o.T)
    wg = pack_ko(w_gate.T)

    perm = np.concatenate([np.arange(0, DH, 2), np.arange(1, DH, 2)])
    rows = np.arange(3 * D)
    qk_perm = rows.copy()
    for h in range(H):
        qk_perm[h * DH:(h + 1) * DH] = h * DH + perm                  # q rows
        qk_perm[D + h * DH:D + (h + 1) * DH] = D + h * DH + perm      # k rows
    wqkvT = w_qkv[qk_perm].T                                          # [1024, 3072]
    wqkv = np.empty((P, KB * 24 * P), dtype=BF)
    for k in range(KB):
        for ot in range(24):
            wqkv[:, (k * 24 + ot) * P:(k * 24 + ot + 1) * P] = \
                wqkvT[k * P:(k + 1) * P, ot * P:(ot + 1) * P]

    inv_freq = 1.0 / (10000.0 ** (np.arange(0, DH, 2, dtype=np.float32) / DH))
    pos = np.arange(T, dtype=np.float32)
    theta = pos[None, :] * inv_freq[:, None]              # [32, T]
    cos64 = np.concatenate([np.cos(theta), np.cos(theta)], axis=0)
    sin64 = np.concatenate([-np.sin(theta), np.sin(theta)], axis=0)
    ctab_full = np.tile(cos64, (2, 1)).astype(BF)         # [128, T]
    stab_full = np.tile(sin64, (2, 1)).astype(BF)

    p64 = np.zeros((P, P), dtype=BF)
    for r in range(P):
        p64[r, (r % 64 + 32) % 64 + 64 * (r // 64)] = 1.0
    mask = np.triu(np.ones((P, P), np.float32))

    wmg = np.empty((P, KB * 4), dtype=BF)
    for k in range(KB):
        wmg[:, k * 4:(k + 1) * 4] = w_mg.T[k * P:(k + 1) * P]

    b1p = np.empty((P, E * 32), dtype=np.float32)
    for e in range(E):
        b1p[:, e * 32:(e + 1) * 32] = b1[e].reshape(32, P).T
    b2xp = np.empty((P, E * 8), dtype=np.float32)
    for e in range(E):
        b2xp[:, e * 8:(e + 1) * 8] = b2[e].reshape(8, P).T * 128.0

    import ml_dtypes as _mld
    F8 = _mld.float8_e4m3

    def q8(a):
        return np.clip(a * 128.0, -240.0, 240.0).astype(F8)

    w1s = np.empty((P, E * 16 * 2048), dtype=F8)
    w2s = np.empty((P, E * 16 * 2048), dtype=F8)
    for e in range(E):
        w1eT = q8(w1[e].T)       # [1024 in, 4096 hid], x128 in fp8
        w2eT = q8(w2[e].T)       # [4096 hid, 1024 out]
        for s in range(16):
            base = (e * 16 + s) * 2048
            for kp in range(4):
                for j in range(2):
                    ht = s * 2 + j
                    for kk in range(2):
                        k = kp * 2 + kk
                        col = base + ((kp * 2 + j) * 2 + kk) * P
                        w1s[:, col:col + P] = \
                            w1eT[k * P:(k + 1) * P, ht * P:(ht + 1) * P]
            for ot in range(8):
                for cc in range(2):
                    ckg = s * 2 + cc
                    col = base + (ot * 2 + cc) * P
                    w2s[:, col:col + P] = \
                        w2eT[ckg * P:(ckg + 1) * P, ot * P:(ot + 1) * P]

    bgp = b_gate.reshape(8, P).T.astype(np.float32)
    lngp = ln_g.reshape(8, P).T.astype(np.float32)
    lnbp = ln_b.reshape(8, P).T.astype(np.float32)
    wdw = np.empty((P, KB * 3), dtype=np.float32)
    for k in range(KB):
        wdw[:, k * 3:(k + 1) * 3] = w_dw[k * P:(k + 1) * P]

    in_maps = []
    for c in range(NC_N):
        s = c * TPC
        xw = np.empty((P, KB * (TPC + 2)), dtype=BF)
        hx = np.zeros((P, KB * 3), dtype=BF)
        xs32 = np.empty((P, KB * TPC), dtype=np.float32)
        for k in range(KB):
            rb = slice(k * P, (k + 1) * P)
            # window tokens s-2 .. s+255  ->  xTp cols s+1 .. s+259
            xw[:, k * (TPC + 2):(k + 1) * (TPC + 2)] = xTp[rb, s + 1:s + TPC + 3]
            xs32[:, k * TPC:(k + 1) * TPC] = xT[rb, s:s + TPC]
            if c == 0:
                # want x1_prev == x1[0]: dw window (0, 0, x0)
                hx[:, k * 3 + 2] = xT[rb, 0]
            else:
                hx[:, k * 3:(k + 1) * 3] = xTp[rb, s:s + 3]
        mask8 = np.zeros((P, 8), dtype=np.float32)
        mask8[:, :c] = 1.0
        sofs = (c * P + np.arange(P, dtype=np.int32)).reshape(P, 1)
        in_maps.append({
            "sofs": sofs,
            "xw": xw, "hx": hx, "xs32": xs32, "wdw": wdw,
            "wpw": wpw, "wqkv": wqkv,
            "ctab": np.ascontiguousarray(ctab_full[:, s:s + TPC]),
            "stab": np.ascontiguousarray(stab_full[:, s:s + TPC]),
            "p64": p64, "mask": mask, "mask8": mask8,
            "wo": wo, "wg": wg, "bg": bgp, "lng": lngp, "lnb": lnbp,
            "wmg": wmg, "b1": b1p, "b2x": b2xp,
            "w1s": w1s, "w2s": w2s,
        })
    return in_maps


def kernel(**inputs) -> np.ndarray:
    global _PROGRAM
    from concourse.bass_utils import run_bass_kernel_spmd

    if _PROGRAM is None:
        _PROGRAM = _build_program()
    nc = _PROGRAM
    in_maps = _prep_inputs(inputs)
    last_err = None
    for _attempt in range(2):
        try:
            res = run_bass_kernel_spmd(nc, in_maps, list(range(NC_N)))
            break
        except Exception as exc:
            last_err = exc
    else:
        raise last_err
    outT = np.empty((D, T), dtype=np.float32)
    for c in range(NC_N):
        y = res.results[c]["y"]                       # [128, 8*256]
        for k in range(KB):
            outT[k * P:(k + 1) * P, c * TPC:(c + 1) * TPC] = \
                y[:, k * TPC:(k + 1) * TPC]
    return np.ascontiguousarray(outT.T)[None, :, :].astype(np.float32)

